# revision 8
# baseline (speedup 1.0000x reference)
"""Trainium2 Bass kernel for DiamondLayer.

Computes out[b, d] = mean(x[b, d:d+16, d+17:d+33]) for d in [0, 2016):
16x16 mean-pool windows sliding along the diagonal of each 2048x2048 matrix.

Sharding: pure data parallel over batch - 32 batches -> 8 cores x 4 batches.

Per-core kernel (raw bacc, no Tile):
  - Only the diagonal band cols [r+2, r+33) of row r is ever touched, so each
    core DMAs just that band with a strided access pattern: partition p holds
    rows [16p, 16p+16), each row one 124B run (2032 descriptors/batch,
    ~1/64 of a full matrix read). One DMA per batch on the SP (sync) HWDGE
    ring; rows 2016..2031 ride in the same AP (their tails wrap in-matrix,
    and the wrapped garbage cancels in the prefix-sum differences).
  - VectorE computes a per-partition prefix scan P of the flat band
    (tensor_tensor_scan); window sums become differences of P. The band's
    unwritten gap columns {0,32,...,512} are zeroed by GPSIMD memsets.
  - An SBUF->SBUF DMA on the ACT (scalar) ring - off the band ring - shifts
    P[q+1, 0:465) into partition q at column 512 ("halo").
  - out[16q+u]*256 = sum_s P[32u+31s+31] - sum_s P[32u+31s+15]: two strided
    VectorE reduces + one fused tensor_tensor_reduce (subtract, scale=1/256),
    self-synchronized on DVE via the vred semaphore (posted writes).
  - Output DMAs go on the SP ring after the band DMAs have been issued.
"""

import os
import sys

import numpy as np

for _p in ("/opt/trn_rl_repo",):
    if _p not in sys.path:
        sys.path.insert(0, _p)

B_FULL = 32
N_CORES = 8
B_PER_CORE = B_FULL // N_CORES  # 4
MAT = 2048
ND = MAT - 32  # 2016
NQ = ND // 16  # 126
NP = NQ + 1  # 127
ROW_STRIDE = MAT + 1  # 2049
MAT_ELEMS = MAT * MAT
BTW = 544  # band buffer pitch (cols 0..512 used)
PW = 1024  # prefix buffer pitch (cols 0..976 used)
HALO = 465  # halo columns: max index 32*15+31*15+31 = 976 -> 976-512+1

LAST_EXEC_TIME_NS = None
_COMPILED = None


def _ensure_axon_ntff_hook():
    """This image's antenv lacks axon_hooks; bass_utils hard-imports it when
    trace=True under axon. Recreate the module and install the ctypes-based
    NTFF hook the boot shim would have installed. Degrades to no-op."""
    try:
        from antenv import axon_hooks  # noqa: F401

        return
    except ImportError:
        pass
    try:
        import types

        import antenv

        m = types.ModuleType("antenv.axon_hooks")
        _hook = [None]
        m.set_axon_ntff_profile_hook = lambda h: _hook.__setitem__(0, h)
        m.get_axon_ntff_profile_hook = lambda: _hook[0]
        sys.modules["antenv.axon_hooks"] = m
        antenv.axon_hooks = m
        if "/root/.axon_site" not in sys.path:
            sys.path.insert(0, "/root/.axon_site")
        from trn_agent_boot import trn_boot

        hook = trn_boot._ntff_profile_via_ctypes("/opt/axon/libaxon_pjrt.so")
        if hook is not None:
            m.set_axon_ntff_profile_hook(hook)
    except Exception:
        pass


def _build():
    import concourse.bass as bass
    import concourse.bacc as bacc
    from concourse import mybir
    from contextlib import ExitStack

    f32 = mybir.dt.float32
    add = mybir.AluOpType.add
    sub_op = mybir.AluOpType.subtract
    bypass = mybir.AluOpType.bypass
    X = mybir.AxisListType.X

    nc = bacc.Bacc("TRN2", target_bir_lowering=False, debug=False)
    x = nc.dram_tensor("x", [B_PER_CORE, MAT, MAT], f32, kind="ExternalInput")
    y = nc.dram_tensor("y", [B_PER_CORE, ND], f32, kind="ExternalOutput")

    def v(t, off, pat):
        return bass.AP(t, off, pat)

    with ExitStack() as ctx:
        B = B_PER_CORE
        e = ctx.enter_context
        bts = [e(nc.sbuf_tensor(f"bt{i}", [NP, BTW], f32)) for i in range(B)]
        pps = [e(nc.sbuf_tensor(f"pp{i}", [NP, PW], f32)) for i in range(B)]
        rs1 = [e(nc.sbuf_tensor(f"r1_{i}", [NQ, 16], f32)) for i in range(B)]
        rs2 = [e(nc.sbuf_tensor(f"r2_{i}", [NQ, 16], f32)) for i in range(B)]
        ros = [e(nc.sbuf_tensor(f"ro{i}", [NQ, 16], f32)) for i in range(B)]
        tmp = [e(nc.sbuf_tensor(f"tm{i}", [NQ, 16], f32)) for i in range(B)]
        bsem = [e(nc.semaphore(f"bsem{i}")) for i in range(B)]
        hsem = [e(nc.semaphore(f"hsem{i}")) for i in range(B)]
        gsem = e(nc.semaphore("gsem"))
        vscan = e(nc.semaphore("vscan"))
        vred = e(nc.semaphore("vred"))
        psem = e(nc.semaphore("psem"))
        vec_done = e(nc.semaphore("vec_done"))
        dma_out = e(nc.semaphore("dma_out"))
        block = e(nc.Block(no_gpsimd_drain=True))

        @block.sync
        def _(sync):
            for b in range(B):
                # band: bt[p, 1+32t+j] = x[b, 16p+t, 16p+t+2+j], j in [0,31)
                sync.dma_start(
                    v(bts[b], 1, [[BTW, NP], [32, 16], [1, 31]]),
                    bass.AP(
                        x,
                        b * MAT_ELEMS + 2,
                        [[16 * ROW_STRIDE, NP], [ROW_STRIDE, 16], [1, 31]],
                    ),
                ).then_inc(bsem[b], 16)
            for b in range(B):
                sync.wait_ge(vec_done, b + 1)
                sync.dma_start(
                    bass.AP(y, b * ND, [[16, NQ], [1, 16]]),
                    v(ros[b], 0, [[16, NQ], [1, 16]]),
                ).then_inc(dma_out, 16)
            sync.wait_ge(dma_out, 16 * B)

        @block.scalar
        def _(scalar):
            for b in range(B):
                # halo: PP[q, 512+g] = P[q+1, g], g in [0, 465)
                scalar.wait_ge(vscan, b + 1)
                scalar.dma_start(
                    v(pps[b], 512, [[PW, NQ], [1, HALO]]),
                    v(pps[b], PW, [[PW, NQ], [1, HALO]]),
                ).then_inc(hsem[b], 16)

        @block.gpsimd
        def _(gpsimd):
            # combine: ro = (rs1 - rs2) / 256, ping-ponged via psem
            for b in range(B):
                gpsimd.wait_ge(vred, 2 * (b + 1))
                nc.gpsimd.tensor_tensor(
                    out=v(tmp[b], 0, [[16, NQ], [1, 16]]),
                    in0=v(rs1[b], 0, [[16, NQ], [1, 16]]),
                    in1=v(rs2[b], 0, [[16, NQ], [1, 16]]),
                    op=sub_op,
                ).then_inc(psem, 1)
            for b in range(B):
                gpsimd.wait_ge(psem, b + 1)
                nc.gpsimd.tensor_scalar_mul(
                    v(ros[b], 0, [[16, NQ], [1, 16]]),
                    v(tmp[b], 0, [[16, NQ], [1, 16]]),
                    1.0 / 256.0,
                ).then_inc(vec_done, 1)

        def rblock(vector, b):
            # out[16q+u]*256 = sum_s P[32u+31s+31] - sum_s P[32u+31s+15]
            vector.wait_ge(hsem[b], 16)
            nc.vector.reduce_sum(
                out=v(rs1[b], 0, [[16, NQ], [1, 16]]),
                in_=v(pps[b], 31, [[PW, NQ], [32, 16], [31, 16]]),
                axis=X,
            ).then_inc(vred, 1)
            nc.vector.reduce_sum(
                out=v(rs2[b], 0, [[16, NQ], [1, 16]]),
                in_=v(pps[b], 15, [[PW, NQ], [32, 16], [31, 16]]),
                axis=X,
            ).then_inc(vred, 1)

        @block.vector
        def _(vector):
            for b in range(B):
                # band gap cols {0, 32, ..., 512}: never DMA'd, finite for scan
                nc.vector.memset(
                    v(bts[b], 0, [[BTW, NP], [32, 17]]), 0.0
                ).then_inc(gsem, 1)
            vector.wait_ge(gsem, B)
            for b in range(B):
                vector.wait_ge(bsem[b], 16)
                # P[f] = prefix sum of the flat band per partition; P[0] = 0
                # (band col 0 is a zeroed gap, so the scan emits P[0] itself)
                nc.vector.tensor_tensor_scan(
                    out=v(pps[b], 0, [[PW, NP], [1, 512]]),
                    data0=v(bts[b], 0, [[BTW, NP], [1, 512]]),
                    data1=v(bts[b], 0, [[BTW, NP], [1, 512]]),
                    initial=0.0,
                    op0=add,
                    op1=bypass,
                ).then_inc(vscan, 1)
                if b == 2:
                    rblock(vector, 0)
                elif b == 3:
                    rblock(vector, 1)
            rblock(vector, 2)
            rblock(vector, 3)

    nc.compile()
    return nc


def _get_compiled():
    global _COMPILED
    if _COMPILED is None:
        _COMPILED = _build()
    return _COMPILED


def kernel(x: np.ndarray) -> np.ndarray:
    global LAST_EXEC_TIME_NS
    from concourse.bass_utils import run_bass_kernel_spmd

    x = np.ascontiguousarray(np.asarray(x), dtype=np.float32)
    assert x.shape == (B_FULL, MAT, MAT), x.shape

    nc = _get_compiled()
    in_maps = [
        {"x": x[i * B_PER_CORE : (i + 1) * B_PER_CORE]} for i in range(N_CORES)
    ]
    trace = bool(int(os.environ.get("KERNEL_TRACE", "0")))
    if trace:
        _ensure_axon_ntff_hook()
        # test-only: keep NTFF artifacts local instead of uploading
        from concourse import bass_utils as _bu

        _bu.upload_artifacts = lambda tmpdir: tmpdir
    res = run_bass_kernel_spmd(
        nc, in_maps, core_ids=list(range(N_CORES)), trace=trace
    )
    LAST_EXEC_TIME_NS = res.exec_time_ns
    out = np.concatenate([res.results[i]["y"] for i in range(N_CORES)], axis=0)
    return out.astype(np.float32)


# revision 12
# speedup vs baseline: 3.2337x; 3.2337x over previous
"""Trainium2 Bass kernel for DiamondLayer.

Computes out[b, d] = mean(x[b, d:d+16, d+17:d+33]) for d in [0, 2016):
16x16 mean-pool windows sliding along the diagonal of each 2048x2048 matrix.

Sharding: pure data parallel over batch - 32 batches -> 8 cores x 4 batches.

Per-core kernel (raw bacc, no Tile):
  - Only the diagonal band cols [r+2, r+33) of row r is ever touched, so each
    core DMAs just that band with a strided access pattern: partition p holds
    rows [16p, 16p+16), each row one 124B run (2032 descriptors/batch,
    ~1/64 of a full matrix read). One DMA per batch on the SP (sync) HWDGE
    ring; rows 2016..2031 ride in the same AP (their tails wrap in-matrix,
    and the wrapped garbage cancels in the prefix-sum differences).
  - VectorE computes a per-partition prefix scan P of the flat band
    (tensor_tensor_scan); window sums become differences of P. The band's
    unwritten gap columns {0,32,...,512} are zeroed by GPSIMD memsets.
  - An SBUF->SBUF DMA on the ACT (scalar) ring - off the band ring - shifts
    P[q+1, 0:465) into partition q at column 512 ("halo").
  - out[16q+u]*256 = sum_s P[32u+31s+31] - sum_s P[32u+31s+15]: two strided
    VectorE reduces + one fused tensor_tensor_reduce (subtract, scale=1/256),
    self-synchronized on DVE via the vred semaphore (posted writes).
  - Output DMAs go on the SP ring after the band DMAs have been issued.
"""

import os
import sys

import numpy as np

for _p in ("/opt/trn_rl_repo",):
    if _p not in sys.path:
        sys.path.insert(0, _p)

B_FULL = 32
N_CORES = 8
B_PER_CORE = B_FULL // N_CORES  # 4
MAT = 2048
ND = MAT - 32  # 2016
NQ = ND // 16  # 126
NP = NQ + 1  # 127
ROW_STRIDE = MAT + 1  # 2049
MAT_ELEMS = MAT * MAT
BTW = 1024  # band buffer pitch (cols 0..512 used)
PW = 1024  # prefix buffer pitch (cols 0..976 used)
HALO = 465  # halo columns: max index 32*15+31*15+31 = 976 -> 976-512+1

LAST_EXEC_TIME_NS = None
_COMPILED = None


def _ensure_axon_ntff_hook():
    """This image's antenv lacks axon_hooks; bass_utils hard-imports it when
    trace=True under axon. Recreate the module and install the ctypes-based
    NTFF hook the boot shim would have installed. Degrades to no-op."""
    try:
        from antenv import axon_hooks  # noqa: F401

        return
    except ImportError:
        pass
    try:
        import types

        import antenv

        m = types.ModuleType("antenv.axon_hooks")
        _hook = [None]
        m.set_axon_ntff_profile_hook = lambda h: _hook.__setitem__(0, h)
        m.get_axon_ntff_profile_hook = lambda: _hook[0]
        sys.modules["antenv.axon_hooks"] = m
        antenv.axon_hooks = m
        if "/root/.axon_site" not in sys.path:
            sys.path.insert(0, "/root/.axon_site")
        from trn_agent_boot import trn_boot

        hook = trn_boot._ntff_profile_via_ctypes("/opt/axon/libaxon_pjrt.so")
        if hook is not None:
            m.set_axon_ntff_profile_hook(hook)
    except Exception:
        pass


def _build():
    import concourse.bass as bass
    import concourse.bacc as bacc
    from concourse import mybir
    from contextlib import ExitStack

    f32 = mybir.dt.float32
    add = mybir.AluOpType.add
    sub_op = mybir.AluOpType.subtract
    bypass = mybir.AluOpType.bypass
    X = mybir.AxisListType.X

    nc = bacc.Bacc("TRN2", target_bir_lowering=False, debug=False)
    x = nc.dram_tensor("x", [B_PER_CORE, MAT, MAT], f32, kind="ExternalInput")
    y = nc.dram_tensor("y", [B_PER_CORE, ND], f32, kind="ExternalOutput")

    def v(t, off, pat):
        return bass.AP(t, off, pat)

    with ExitStack() as ctx:
        B = B_PER_CORE
        e = ctx.enter_context
        bts = [e(nc.sbuf_tensor(f"bt{i}", [NP, BTW], f32)) for i in range(B)]
        pps = [e(nc.sbuf_tensor(f"pp{i}", [NP, PW], f32)) for i in range(B)]
        rs1 = [e(nc.sbuf_tensor(f"r1_{i}", [NQ, 16], f32)) for i in range(B)]
        rs2 = [e(nc.sbuf_tensor(f"r2_{i}", [NQ, 16], f32)) for i in range(B)]
        ros = [e(nc.sbuf_tensor(f"ro{i}", [NQ, 16], f32)) for i in range(B)]
        tmp = [e(nc.sbuf_tensor(f"tm{i}", [NQ, 16], f32)) for i in range(B)]
        bsem = [e(nc.semaphore(f"bsem{i}")) for i in range(B)]
        tsem = [e(nc.semaphore(f"tsem{i}")) for i in range(B)]
        hsem = [e(nc.semaphore(f"hsem{i}")) for i in range(B)]
        gsem = e(nc.semaphore("gsem"))
        vscan = e(nc.semaphore("vscan"))
        vred = e(nc.semaphore("vred"))
        psem = e(nc.semaphore("psem"))
        vec_done = e(nc.semaphore("vec_done"))
        dma_out = e(nc.semaphore("dma_out"))
        block = e(nc.Block(no_gpsimd_drain=True))

        @block.sync
        def _(sync):
            for b in range(B):
                # band: bt[p, 1+32t+j] = x[b, 16p+t, 16p+t+2+j], j in [0,32)
                # (126 partitions here; partition 126 rides the scalar ring -
                # this shape is what makes HWDGE spray descriptors across all
                # 16 SDMA engines; a 127-partition/31-col variant lands on ONE)
                sync.dma_start(
                    v(bts[b], 1, [[BTW, NQ], [32, 16], [1, 32]]),
                    bass.AP(
                        x,
                        b * MAT_ELEMS + 2,
                        [[16 * ROW_STRIDE, NQ], [ROW_STRIDE, 16], [1, 32]],
                    ),
                ).then_inc(bsem[b], 16)
            for b in range(B):
                sync.wait_ge(vec_done, b + 1)
                sync.dma_start(
                    bass.AP(y, b * ND, [[16, NQ], [1, 16]]),
                    v(ros[b], 0, [[16, NQ], [1, 16]]),
                ).then_inc(dma_out, 16)
            sync.wait_ge(dma_out, 16 * B)

        @block.scalar
        def _(scalar):
            for b in range(B):
                # partition 126's band rows (halo source for q=125)
                scalar.dma_start(
                    v(bts[b], NQ * BTW + 1, [[BTW, 1], [32, 16], [1, 32]]),
                    bass.AP(
                        x,
                        b * MAT_ELEMS + 2 + NQ * 16 * ROW_STRIDE,
                        [[16 * ROW_STRIDE, 1], [ROW_STRIDE, 16], [1, 32]],
                    ),
                ).then_inc(tsem[b], 16)
            for b in range(B):
                # halo: PP[q, 512+g] = P[q+1, g], g in [0, 465)
                scalar.wait_ge(vscan, b + 1)
                scalar.dma_start(
                    v(pps[b], 512, [[PW, NQ], [1, HALO]]),
                    v(pps[b], PW, [[PW, NQ], [1, HALO]]),
                ).then_inc(hsem[b], 16)

        @block.gpsimd
        def _(gpsimd):
            # combine: ro = (rs1 - rs2) / 256, ping-ponged via psem
            for b in range(B):
                gpsimd.wait_ge(vred, 2 * (b + 1))
                nc.gpsimd.tensor_tensor(
                    out=v(tmp[b], 0, [[16, NQ], [1, 16]]),
                    in0=v(rs1[b], 0, [[16, NQ], [1, 16]]),
                    in1=v(rs2[b], 0, [[16, NQ], [1, 16]]),
                    op=sub_op,
                ).then_inc(psem, 1)
            for b in range(B):
                gpsimd.wait_ge(psem, b + 1)
                nc.gpsimd.tensor_scalar_mul(
                    v(ros[b], 0, [[16, NQ], [1, 16]]),
                    v(tmp[b], 0, [[16, NQ], [1, 16]]),
                    1.0 / 256.0,
                ).then_inc(vec_done, 1)

        def rblock(vector, b):
            # out[16q+u]*256 = sum_s P[32u+31s+31] - sum_s P[32u+31s+15]
            vector.wait_ge(hsem[b], 16)
            nc.vector.reduce_sum(
                out=v(rs1[b], 0, [[16, NQ], [1, 16]]),
                in_=v(pps[b], 31, [[PW, NQ], [32, 16], [31, 16]]),
                axis=X,
            ).then_inc(vred, 1)
            nc.vector.reduce_sum(
                out=v(rs2[b], 0, [[16, NQ], [1, 16]]),
                in_=v(pps[b], 15, [[PW, NQ], [32, 16], [31, 16]]),
                axis=X,
            ).then_inc(vred, 1)

        @block.vector
        def _(vector):
            for b in range(B):
                # band col 0: never DMA'd; zero so the scan emits P[0] = 0
                nc.vector.memset(
                    v(bts[b], 0, [[BTW, NP], [1, 1]]), 0.0
                ).then_inc(gsem, 1)
            vector.wait_ge(gsem, B)
            for b in range(B):
                vector.wait_ge(bsem[b], 16)
                vector.wait_ge(tsem[b], 16)
                # P[f] = prefix sum of the flat band per partition; P[0] = 0
                nc.vector.tensor_tensor_scan(
                    out=v(pps[b], 0, [[PW, NP], [1, 513]]),
                    data0=v(bts[b], 0, [[BTW, NP], [1, 513]]),
                    data1=v(bts[b], 0, [[BTW, NP], [1, 513]]),
                    initial=0.0,
                    op0=add,
                    op1=bypass,
                ).then_inc(vscan, 1)
                if b == 2:
                    rblock(vector, 0)
                elif b == 3:
                    rblock(vector, 1)
            rblock(vector, 2)
            rblock(vector, 3)

    nc.compile()
    return nc


def _get_compiled():
    global _COMPILED
    if _COMPILED is None:
        _COMPILED = _build()
    return _COMPILED


def kernel(x: np.ndarray) -> np.ndarray:
    global LAST_EXEC_TIME_NS
    from concourse.bass_utils import run_bass_kernel_spmd

    x = np.ascontiguousarray(np.asarray(x), dtype=np.float32)
    assert x.shape == (B_FULL, MAT, MAT), x.shape

    nc = _get_compiled()
    in_maps = [
        {"x": x[i * B_PER_CORE : (i + 1) * B_PER_CORE]} for i in range(N_CORES)
    ]
    trace = bool(int(os.environ.get("KERNEL_TRACE", "0")))
    if trace:
        _ensure_axon_ntff_hook()
        # test-only: keep NTFF artifacts local instead of uploading
        from concourse import bass_utils as _bu

        _bu.upload_artifacts = lambda tmpdir: tmpdir
    res = run_bass_kernel_spmd(
        nc, in_maps, core_ids=list(range(N_CORES)), trace=trace
    )
    LAST_EXEC_TIME_NS = res.exec_time_ns
    out = np.concatenate([res.results[i]["y"] for i in range(N_CORES)], axis=0)
    return out.astype(np.float32)


# revision 19
# speedup vs baseline: 3.5803x; 1.1072x over previous
"""Trainium2 Bass kernel for DiamondLayer.

Computes out[b, d] = mean(x[b, d:d+16, d+17:d+33]) for d in [0, 2016):
16x16 mean-pool windows sliding along the diagonal of each 2048x2048 matrix.

Sharding: pure data parallel over batch - 32 batches -> 8 cores x 4 batches.

Per-core kernel (raw bacc, no Tile):
  - Only the diagonal band cols [r+2, r+34) of row r is ever touched, so each
    core DMAs just that band with a strided access pattern: partition p holds
    rows [16p, 16p+16), one 128B run per row (2016+16 descriptors/batch).
    One band DMA per batch on the SP ring (126 partitions - the HWDGE spray
    across 14 SDMA engines needs the outer AP count divisible by 14) plus a
    partition-126 tail DMA on the ACT ring.
  - VectorE computes a per-partition prefix scan P of the flat band
    (tensor_tensor_scan); window sums become differences of P.
  - PE (tensor engine) builds the halo'd, prescaled prefix buffer in PSUM:
    matmul with (1/256)*I copies P[q, 0:512) to bank 0, matmul with a
    (1/256)*shift matrix copies P[q+1, 0:465) to bank 1 - replacing the
    SBUF->SBUF halo DMA (which used to steal SDMA descriptor throughput
    from the band and cost ~3us of tail latency).
  - out[16q+u] = sum_s PPH[32u+31s+31] - sum_s PPH[32u+31s+15] with
    PPH = P/256 from PSUM: one strided reduce on DVE (R1), one on GPSIMD
    (R2), then a DVE subtract. The 1/256 rides in the matmul weights.
  - One merged output DMA (all 4 batches) on the SP ring.
"""

import os
import sys

import numpy as np

for _p in ("/opt/trn_rl_repo",):
    if _p not in sys.path:
        sys.path.insert(0, _p)

B_FULL = 32
N_CORES = 8
B_PER_CORE = B_FULL // N_CORES  # 4
MAT = 2048
ND = MAT - 32  # 2016
NQ = ND // 16  # 126
NP = NQ + 1  # 127
ROW_STRIDE = MAT + 1  # 2049
MAT_ELEMS = MAT * MAT
BTW = 1024  # band buffer pitch (cols 0..512 used)
PPW = 544  # prefix buffer pitch (cols 0..512 used)
HALO = 465  # halo columns: max index 32*15+31*15+31 = 976 -> 976-512+1

LAST_EXEC_TIME_NS = None
_COMPILED = None


def _ensure_axon_ntff_hook():
    """This image's antenv lacks axon_hooks; bass_utils hard-imports it when
    trace=True under axon. Recreate the module and install the ctypes-based
    NTFF hook the boot shim would have installed. Degrades to no-op."""
    try:
        from antenv import axon_hooks  # noqa: F401

        return
    except ImportError:
        pass
    try:
        import types

        import antenv

        m = types.ModuleType("antenv.axon_hooks")
        _hook = [None]
        m.set_axon_ntff_profile_hook = lambda h: _hook.__setitem__(0, h)
        m.get_axon_ntff_profile_hook = lambda: _hook[0]
        sys.modules["antenv.axon_hooks"] = m
        antenv.axon_hooks = m
        if "/root/.axon_site" not in sys.path:
            sys.path.insert(0, "/root/.axon_site")
        from trn_agent_boot import trn_boot

        hook = trn_boot._ntff_profile_via_ctypes("/opt/axon/libaxon_pjrt.so")
        if hook is not None:
            m.set_axon_ntff_profile_hook(hook)
    except Exception:
        pass


def _make_weights() -> np.ndarray:
    """[127, 256] f32: cols 0..125 = (1/256)*I (PE copy of P[q]),
    cols 128..253 = (1/256)*shift (PE copy of P[q+1])."""
    w = np.zeros((NP, 256), dtype=np.float32)
    w[np.arange(NQ), np.arange(NQ)] = 1.0 / 256.0
    w[np.arange(1, NP), 128 + np.arange(NQ)] = 1.0 / 256.0
    return w


def _build():
    import concourse.bass as bass
    import concourse.bacc as bacc
    from concourse import mybir
    from contextlib import ExitStack

    f32 = mybir.dt.float32
    add = mybir.AluOpType.add
    sub_op = mybir.AluOpType.subtract
    bypass = mybir.AluOpType.bypass
    X = mybir.AxisListType.X

    nc = bacc.Bacc("TRN2", target_bir_lowering=False, debug=False)
    x = nc.dram_tensor("x", [B_PER_CORE, MAT, MAT], f32, kind="ExternalInput")
    w = nc.dram_tensor("w", [NP, 256], f32, kind="ExternalInput")
    y = nc.dram_tensor("y", [B_PER_CORE, ND], f32, kind="ExternalOutput")

    def v(t, off, pat):
        return bass.AP(t, off, pat)

    with ExitStack() as ctx:
        B = B_PER_CORE
        e = ctx.enter_context
        bts = [e(nc.sbuf_tensor(f"bt{i}", [NP, BTW], f32)) for i in range(B)]
        pps = [e(nc.sbuf_tensor(f"pp{i}", [NP, PPW], f32)) for i in range(B)]
        wt = e(nc.sbuf_tensor("wt", [NP, 256], f32))
        rs1 = [e(nc.sbuf_tensor(f"r1_{i}", [NQ, 16], f32)) for i in range(B)]
        rs2 = [e(nc.sbuf_tensor(f"r2_{i}", [NQ, 16], f32)) for i in range(B)]
        ro = e(nc.sbuf_tensor("ro", [NQ, 64], f32))
        pph = [nc.alloc_psum_tensor(f"ph{i}", [NQ, 1024], f32) for i in range(B)]
        bsem = [e(nc.semaphore(f"bsem{i}")) for i in range(B)]
        tsem = [e(nc.semaphore(f"tsem{i}")) for i in range(B)]
        wsem = e(nc.semaphore("wsem"))
        gsem = e(nc.semaphore("gsem"))
        vscan = e(nc.semaphore("vscan"))
        mmsem = e(nc.semaphore("mmsem"))
        vred = e(nc.semaphore("vred"))
        vec_done = e(nc.semaphore("vec_done"))
        dma_out = e(nc.semaphore("dma_out"))
        block = e(nc.Block(no_gpsimd_drain=True))

        @block.sync
        def _(sync):
            for b in range(B):
                # band: bt[p, 1+32t+j] = x[b, 16p+t, 16p+t+2+j], j in [0,32)
                sync.dma_start(
                    v(bts[b], 1, [[BTW, NQ], [32, 16], [1, 32]]),
                    bass.AP(
                        x,
                        b * MAT_ELEMS + 2,
                        [[16 * ROW_STRIDE, NQ], [ROW_STRIDE, 16], [1, 32]],
                    ),
                ).then_inc(bsem[b], 16)
            # merged output: y[b, 16q+u] <- ro[q, 16b+u], all batches
            sync.wait_ge(vec_done, B)
            sync.dma_start(
                bass.AP(y, 0, [[16, NQ], [ND, B], [1, 16]]),
                v(ro, 0, [[64, NQ], [16, B], [1, 16]]),
            ).then_inc(dma_out, 16)
            sync.wait_ge(dma_out, 16)

        @block.scalar
        def _(scalar):
            # split 126+1 so the big piece's outer count stays spray-friendly
            scalar.dma_start(
                v(wt, 0, [[256, NQ], [1, 256]]),
                bass.AP(w, 0, [[256, NQ], [1, 256]]),
            ).then_inc(wsem, 16)
            scalar.dma_start(
                v(wt, NQ * 256, [[256, 1], [1, 256]]),
                bass.AP(w, NQ * 256, [[256, 1], [1, 256]]),
            ).then_inc(wsem, 16)
            for b in range(B):
                # partition 126's band rows (halo source for q=125)
                scalar.dma_start(
                    v(bts[b], NQ * BTW + 1, [[BTW, 1], [32, 16], [1, 32]]),
                    bass.AP(
                        x,
                        b * MAT_ELEMS + 2 + NQ * 16 * ROW_STRIDE,
                        [[16 * ROW_STRIDE, 1], [ROW_STRIDE, 16], [1, 32]],
                    ),
                ).then_inc(tsem[b], 16)

        @block.tensor
        def _(tensor):
            tensor.wait_ge(wsem, 32)
            for b in range(B):
                tensor.wait_ge(vscan, b + 1)
                # PPH[q, f] = P[q, f]/256 for f in [0, 512)  (PSUM bank 0)
                nc.tensor.matmul(
                    v(pph[b], 0, [[1024, NQ], [1, 512]]),
                    v(wt, 0, [[256, NP], [1, NQ]]),
                    v(pps[b], 0, [[PPW, NP], [1, 512]]),
                    start=True,
                    stop=True,
                ).then_inc(mmsem, 1)
                # PPH[q, 512+g] = P[q+1, g]/256 for g in [0, 465) (bank 1)
                nc.tensor.matmul(
                    v(pph[b], 512, [[1024, NQ], [1, HALO]]),
                    v(wt, 128, [[256, NP], [1, NQ]]),
                    v(pps[b], 0, [[PPW, NP], [1, HALO]]),
                    start=True,
                    stop=True,
                ).then_inc(mmsem, 1)

        def rred(vector, b, which):
            # R1[q,u] = sum_s PPH[q, 32u+31s+31]; R2: base 15
            vector.wait_ge(mmsem, 2 * (b + 1))
            dst, base = (rs1[b], 31) if which == 1 else (rs2[b], 15)
            nc.vector.reduce_sum(
                out=v(dst, 0, [[16, NQ], [1, 16]]),
                in_=v(pph[b], base, [[1024, NQ], [32, 16], [31, 16]]),
                axis=X,
            ).then_inc(vred, 1)

        @block.vector
        def _(vector):
            for b in range(B):
                # band col 0: never DMA'd; zero so the scan emits P[0] = 0
                nc.vector.memset(
                    v(bts[b], 0, [[BTW, NP], [1, 1]]), 0.0
                ).then_inc(gsem, 1)
            vector.wait_ge(gsem, B)
            for b in range(B):
                vector.wait_ge(bsem[b], 16)
                vector.wait_ge(tsem[b], 16)
                # P[f] = prefix sum of the flat band per partition; P[0] = 0
                nc.vector.tensor_tensor_scan(
                    out=v(pps[b], 0, [[PPW, NP], [1, 513]]),
                    data0=v(bts[b], 0, [[BTW, NP], [1, 513]]),
                    data1=v(bts[b], 0, [[BTW, NP], [1, 513]]),
                    initial=0.0,
                    op0=add,
                    op1=bypass,
                ).then_inc(vscan, 1)
                if b == 2:
                    rred(vector, 0, 1)
                elif b == 3:
                    rred(vector, 0, 2)
            for b in range(1, B):
                rred(vector, b, 1)
                rred(vector, b, 2)
            for b in range(B):
                # out = R1 - R2 (the 1/256 rides in the matmul weights)
                vector.wait_ge(vred, 2 * (b + 1))
                nc.vector.tensor_tensor(
                    out=v(ro, 16 * b, [[64, NQ], [1, 16]]),
                    in0=v(rs1[b], 0, [[16, NQ], [1, 16]]),
                    in1=v(rs2[b], 0, [[16, NQ], [1, 16]]),
                    op=sub_op,
                ).then_inc(vec_done, 1)

    nc.compile()
    return nc


def _get_compiled():
    global _COMPILED
    if _COMPILED is None:
        _COMPILED = _build()
    return _COMPILED


def kernel(x: np.ndarray) -> np.ndarray:
    global LAST_EXEC_TIME_NS
    from concourse.bass_utils import run_bass_kernel_spmd

    x = np.ascontiguousarray(np.asarray(x), dtype=np.float32)
    assert x.shape == (B_FULL, MAT, MAT), x.shape

    nc = _get_compiled()
    wmat = _make_weights()
    in_maps = [
        {"x": x[i * B_PER_CORE : (i + 1) * B_PER_CORE], "w": wmat}
        for i in range(N_CORES)
    ]
    trace = bool(int(os.environ.get("KERNEL_TRACE", "0")))
    if trace:
        _ensure_axon_ntff_hook()
        # test-only: keep NTFF artifacts local instead of uploading
        from concourse import bass_utils as _bu

        _bu.upload_artifacts = lambda tmpdir: tmpdir
    res = run_bass_kernel_spmd(
        nc, in_maps, core_ids=list(range(N_CORES)), trace=trace
    )
    LAST_EXEC_TIME_NS = res.exec_time_ns
    out = np.concatenate([res.results[i]["y"] for i in range(N_CORES)], axis=0)
    return out.astype(np.float32)


# revision 37
# speedup vs baseline: 3.6406x; 1.0168x over previous
"""Trainium2 Bass kernel for DiamondLayer.

Computes out[b, d] = mean(x[b, d:d+16, d+17:d+33]) for d in [0, 2016):
16x16 mean-pool windows sliding along the diagonal of each 2048x2048 matrix.

Sharding: pure data parallel over batch - 32 batches -> 8 cores x 4 batches.

Per-core kernel (raw bacc, no Tile):
  - Only the diagonal band cols [r+2, r+34) of row r is ever touched, so each
    core DMAs just that band with a strided access pattern: partition p holds
    rows [16p, 16p+16), one 128B run per row (2016+16 descriptors/batch).
    One band DMA per batch on the SP ring (126 partitions - the HWDGE spray
    across 14 SDMA engines needs the outer AP count divisible by 14) plus a
    partition-126 tail DMA on the ACT ring.
  - VectorE computes a per-partition prefix scan P of the flat band
    (tensor_tensor_scan); window sums become differences of P.
  - The halo'd, prescaled prefix buffer PPH = P/256 is built in PSUM by two
    idle engines: ACT copies P[q, 0:512) to bank 0 (activation Copy with
    scale=1/256) and PE copies P[q+1, 0:465) to bank 1 via a matmul with a
    (1/256)*shift-by-one weight matrix (fed as an extra kernel input) -
    replacing the SBUF->SBUF halo DMA, which used to steal SDMA descriptor
    throughput from the band and cost ~3us of tail latency. Junk warmup
    matmuls release the PE HAM clock throttle first.
  - out[16q+u] = sum_s PPH[32u+31s+31] - sum_s PPH[32u+31s+15]: two strided
    DVE reduces + a DVE subtract per batch (1/256 already applied).
  - Two merged output DMAs on the SP ring (batches 0-2, then batch 3).
"""

import os
import sys

import numpy as np

for _p in ("/opt/trn_rl_repo",):
    if _p not in sys.path:
        sys.path.insert(0, _p)

B_FULL = 32
N_CORES = 8
B_PER_CORE = B_FULL // N_CORES  # 4
MAT = 2048
ND = MAT - 32  # 2016
NQ = ND // 16  # 126
NP = NQ + 1  # 127
ROW_STRIDE = MAT + 1  # 2049
MAT_ELEMS = MAT * MAT
BTW = 1024  # band buffer pitch (cols 0..512 used)
PPW = 544  # prefix buffer pitch (cols 0..512 used)
HALO = 465  # halo columns: max index 32*15+31*15+31 = 976 -> 976-512+1

LAST_EXEC_TIME_NS = None
_COMPILED = None


def _ensure_axon_ntff_hook():
    """This image's antenv lacks axon_hooks; bass_utils hard-imports it when
    trace=True under axon. Recreate the module and install the ctypes-based
    NTFF hook the boot shim would have installed. Degrades to no-op."""
    try:
        from antenv import axon_hooks  # noqa: F401

        return
    except ImportError:
        pass
    try:
        import types

        import antenv

        m = types.ModuleType("antenv.axon_hooks")
        _hook = [None]
        m.set_axon_ntff_profile_hook = lambda h: _hook.__setitem__(0, h)
        m.get_axon_ntff_profile_hook = lambda: _hook[0]
        sys.modules["antenv.axon_hooks"] = m
        antenv.axon_hooks = m
        if "/root/.axon_site" not in sys.path:
            sys.path.insert(0, "/root/.axon_site")
        from trn_agent_boot import trn_boot

        hook = trn_boot._ntff_profile_via_ctypes("/opt/axon/libaxon_pjrt.so")
        if hook is not None:
            m.set_axon_ntff_profile_hook(hook)
    except Exception:
        pass


def _make_weights() -> np.ndarray:
    """[127, 256] f32: cols 0..125 = (1/256)*I (PE copy of P[q]),
    cols 128..253 = (1/256)*shift (copy of P[q+1]; shift[p,oc]=1 iff p==oc+1)."""
    w = np.zeros((NP, 256), dtype=np.float32)
    q = np.arange(NQ)
    w[q, q] = 1.0 / 256.0
    w[q + 1, 128 + q] = 1.0 / 256.0
    return w


def _build():
    import concourse.bass as bass
    import concourse.bacc as bacc
    from concourse import mybir
    from contextlib import ExitStack

    f32 = mybir.dt.float32
    add = mybir.AluOpType.add
    sub_op = mybir.AluOpType.subtract
    bypass = mybir.AluOpType.bypass
    X = mybir.AxisListType.X

    nc = bacc.Bacc("TRN2", target_bir_lowering=False, debug=False)
    x = nc.dram_tensor("x", [B_PER_CORE, MAT, MAT], f32, kind="ExternalInput")
    w = nc.dram_tensor("w", [NP, 256], f32, kind="ExternalInput")
    y = nc.dram_tensor("y", [B_PER_CORE, ND], f32, kind="ExternalOutput")

    def v(t, off, pat):
        return bass.AP(t, off, pat)

    with ExitStack() as ctx:
        B = B_PER_CORE
        e = ctx.enter_context
        bts = [e(nc.sbuf_tensor(f"bt{i}", [NP, BTW], f32)) for i in range(B)]
        pps = [e(nc.sbuf_tensor(f"pp{i}", [NP, PPW], f32)) for i in range(B)]
        wt = e(nc.sbuf_tensor("wt", [NP, 256], f32))
        ro = e(nc.sbuf_tensor("ro", [NQ, 64], f32))
        jnk = e(nc.sbuf_tensor("jnk", [NP, 128], f32))
        rs1 = [e(nc.sbuf_tensor(f"r1_{i}", [NQ, 16], f32)) for i in range(B)]
        rs2 = [e(nc.sbuf_tensor(f"r2_{i}", [NQ, 16], f32)) for i in range(B)]
        pph = [nc.alloc_psum_tensor(f"ph{i}", [NQ, 1024], f32) for i in range(B)]
        bsem = [e(nc.semaphore(f"bsem{i}")) for i in range(B)]
        tsem = [e(nc.semaphore(f"tsem{i}")) for i in range(B)]
        wsem = e(nc.semaphore("wsem"))
        gsem = e(nc.semaphore("gsem"))
        vscan = e(nc.semaphore("vscan"))
        mmsem = e(nc.semaphore("mmsem"))
        acsem = e(nc.semaphore("acsem"))
        vred = e(nc.semaphore("vred"))
        vec_done = e(nc.semaphore("vec_done"))
        dma_out = e(nc.semaphore("dma_out"))
        block = e(nc.Block(no_gpsimd_drain=True))

        @block.sync
        def _(sync):
            for b in range(B):
                # band: bt[p, 1+32t+j] = x[b, 16p+t, 16p+t+2+j], j in [0,32)
                sync.dma_start(
                    v(bts[b], 1, [[BTW, NQ], [32, 16], [1, 32]]),
                    bass.AP(
                        x,
                        b * MAT_ELEMS + 2,
                        [[16 * ROW_STRIDE, NQ], [ROW_STRIDE, 16], [1, 32]],
                    ),
                ).then_inc(bsem[b], 16)
            # weight load after the bands: frees early SDMA throughput; the
            # PE's first real matmul doesn't need wt until well past this
            # (126+1 split keeps the big piece's outer count spray-friendly)
            sync.dma_start(
                v(wt, 0, [[256, NQ], [1, 256]]),
                bass.AP(w, 0, [[256, NQ], [1, 256]]),
            ).then_inc(wsem, 16)
            sync.dma_start(
                v(wt, NQ * 256, [[256, 1], [1, 256]]),
                bass.AP(w, NQ * 256, [[256, 1], [1, 256]]),
            ).then_inc(wsem, 16)
            # merged outputs: y[b, 16q+u] <- ro[q, 16b+u]; batch 3 separate
            # so batches 0-2 overlap the tail
            sync.wait_ge(vec_done, B - 1)
            sync.dma_start(
                bass.AP(y, 0, [[16, NQ], [ND, B - 1], [1, 16]]),
                v(ro, 0, [[64, NQ], [16, B - 1], [1, 16]]),
            ).then_inc(dma_out, 16)
            sync.wait_ge(vec_done, B)
            sync.dma_start(
                bass.AP(y, (B - 1) * ND, [[16, NQ], [1, 16]]),
                v(ro, 16 * (B - 1), [[64, NQ], [1, 16]]),
            ).then_inc(dma_out, 16)
            sync.wait_ge(dma_out, 32)

        @block.scalar
        def _(scalar):
            for b in range(B):
                # partition 126's band rows (halo source for q=125)
                scalar.dma_start(
                    v(bts[b], NQ * BTW + 1, [[BTW, 1], [32, 16], [1, 32]]),
                    bass.AP(
                        x,
                        b * MAT_ELEMS + 2 + NQ * 16 * ROW_STRIDE,
                        [[16 * ROW_STRIDE, 1], [ROW_STRIDE, 16], [1, 32]],
                    ),
                ).then_inc(tsem[b], 16)
            for b in range(B):
                # PPH[q, f] = P[q, f]/256 (PSUM bank 0) on the idle ACT engine
                scalar.wait_ge(vscan, b + 1)
                nc.scalar.activation(
                    out=v(pph[b], 0, [[1024, NQ], [1, 512]]),
                    in_=v(pps[b], 0, [[PPW, NQ], [1, 512]]),
                    func=mybir.ActivationFunctionType.Copy,
                    bias=0.0,
                    scale=1.0 / 256.0,
                ).then_inc(acsem, 1)

        @block.tensor
        def _(tensor):
            # PPH[q, 512+g] = P[q+1, g]/256 (bank 1): the PE's shift matmul
            # replaces the SBUF->SBUF halo DMA; junk warmup matmuls first to
            # release the HAM clock throttle before the real ones arrive.
            tensor.wait_ge(gsem, 1)
            for _ in range(6):
                nc.tensor.matmul(
                    v(pph[0], 512, [[1024, NQ], [1, 64]]),
                    v(jnk, 0, [[128, NP], [1, NQ]]),
                    v(jnk, 0, [[128, NP], [1, 64]]),
                    start=True,
                    stop=True,
                )
            tensor.wait_ge(wsem, 32)
            for b in range(B):
                tensor.wait_ge(vscan, b + 1)
                nc.tensor.matmul(
                    v(pph[b], 512, [[1024, NQ], [1, HALO]]),
                    v(wt, 128, [[256, NP], [1, NQ]]),
                    v(pps[b], 0, [[PPW, NP], [1, HALO]]),
                    start=True,
                    stop=True,
                ).then_inc(mmsem, 1)

        def rblk(vector, b):
            # out[16q+u] = sum_s PPH[32u+31s+31] - sum_s PPH[32u+31s+15]
            vector.wait_ge(mmsem, b + 1)
            vector.wait_ge(acsem, b + 1)
            nc.vector.reduce_sum(
                out=v(rs1[b], 0, [[16, NQ], [1, 16]]),
                in_=v(pph[b], 31, [[1024, NQ], [32, 16], [31, 16]]),
                axis=X,
            ).then_inc(vred, 1)
            nc.vector.reduce_sum(
                out=v(rs2[b], 0, [[16, NQ], [1, 16]]),
                in_=v(pph[b], 15, [[1024, NQ], [32, 16], [31, 16]]),
                axis=X,
            ).then_inc(vred, 1)
            vector.wait_ge(vred, 2 * (b + 1))
            nc.vector.tensor_tensor(
                out=v(ro, 16 * b, [[64, NQ], [1, 16]]),
                in0=v(rs1[b], 0, [[16, NQ], [1, 16]]),
                in1=v(rs2[b], 0, [[16, NQ], [1, 16]]),
                op=sub_op,
            ).then_inc(vec_done, 1)

        @block.vector
        def _(vector):
            # junk operand for the PE warmup matmuls
            nc.vector.memset(v(jnk, 0, [[128, NP], [1, 128]]), 0.0).then_inc(
                gsem, 1
            )
            for b in range(B):
                # band col 0: never DMA'd; zero so the scan emits P[0] = 0
                nc.vector.memset(
                    v(bts[b], 0, [[BTW, NP], [1, 1]]), 0.0
                ).then_inc(gsem, 1)
            vector.wait_ge(gsem, B + 1)
            for b in range(B):
                vector.wait_ge(bsem[b], 16)
                vector.wait_ge(tsem[b], 16)
                # P[f] = prefix sum of the flat band per partition; P[0] = 0
                nc.vector.tensor_tensor_scan(
                    out=v(pps[b], 0, [[PPW, NP], [1, 513]]),
                    data0=v(bts[b], 0, [[BTW, NP], [1, 513]]),
                    data1=v(bts[b], 0, [[BTW, NP], [1, 513]]),
                    initial=0.0,
                    op0=add,
                    op1=bypass,
                ).then_inc(vscan, 1)
                if b == 3:
                    rblk(vector, 0)
            rblk(vector, 1)
            rblk(vector, 2)
            rblk(vector, 3)

    nc.compile()
    return nc


def _get_compiled():
    global _COMPILED
    if _COMPILED is None:
        _COMPILED = _build()
    return _COMPILED


def kernel(x: np.ndarray) -> np.ndarray:
    global LAST_EXEC_TIME_NS
    from concourse.bass_utils import run_bass_kernel_spmd

    x = np.ascontiguousarray(np.asarray(x), dtype=np.float32)
    assert x.shape == (B_FULL, MAT, MAT), x.shape

    nc = _get_compiled()
    wmat = _make_weights()
    in_maps = [
        {"x": x[i * B_PER_CORE : (i + 1) * B_PER_CORE], "w": wmat}
        for i in range(N_CORES)
    ]
    trace = bool(int(os.environ.get("KERNEL_TRACE", "0")))
    if trace:
        _ensure_axon_ntff_hook()
        # test-only: keep NTFF artifacts local instead of uploading
        from concourse import bass_utils as _bu

        _bu.upload_artifacts = lambda tmpdir: tmpdir
    res = run_bass_kernel_spmd(
        nc, in_maps, core_ids=list(range(N_CORES)), trace=trace
    )
    LAST_EXEC_TIME_NS = res.exec_time_ns
    out = np.concatenate([res.results[i]["y"] for i in range(N_CORES)], axis=0)
    return out.astype(np.float32)


# revision 38
# speedup vs baseline: 3.6943x; 1.0148x over previous
"""Trainium2 Bass kernel for DiamondLayer.

Computes out[b, d] = mean(x[b, d:d+16, d+17:d+33]) for d in [0, 2016):
16x16 mean-pool windows sliding along the diagonal of each 2048x2048 matrix.

Sharding: pure data parallel over batch - 32 batches -> 8 cores x 4 batches.

Per-core kernel (raw bacc, no Tile):
  - Only the diagonal band cols [r+2, r+34) of row r is ever touched, so each
    core DMAs just that band with a strided access pattern: partition p holds
    rows [16p, 16p+16), one 128B run per row (2016+16 descriptors/batch).
    One band DMA per batch on the SP ring (126 partitions - the HWDGE spray
    across 14 SDMA engines needs the outer AP count divisible by 14) plus a
    partition-126 tail DMA on the ACT ring.
  - VectorE computes a per-partition prefix scan P of the flat band
    (tensor_tensor_scan); window sums become differences of P.
  - The halo'd, prescaled prefix buffer PPH = P/256 is built in PSUM by two
    idle engines: ACT copies P[q, 0:512) to bank 0 (activation Copy with
    scale=1/256) and PE copies P[q+1, 0:465) to bank 1 via a matmul with a
    (1/256)*shift-by-one weight matrix (fed as an extra kernel input) -
    replacing the SBUF->SBUF halo DMA, which used to steal SDMA descriptor
    throughput from the band and cost ~3us of tail latency. Junk warmup
    matmuls release the PE HAM clock throttle first.
  - out[16q+u] = sum_s PPH[32u+31s+31] - sum_s PPH[32u+31s+15]: two strided
    DVE reduces + a DVE subtract per batch (1/256 already applied).
  - Two merged output DMAs on the SP ring (batches 0-2, then batch 3).
"""

import os
import sys

import numpy as np

for _p in ("/opt/trn_rl_repo",):
    if _p not in sys.path:
        sys.path.insert(0, _p)

B_FULL = 32
N_CORES = 8
B_PER_CORE = B_FULL // N_CORES  # 4
MAT = 2048
ND = MAT - 32  # 2016
NQ = ND // 16  # 126
NP = NQ + 1  # 127
ROW_STRIDE = MAT + 1  # 2049
MAT_ELEMS = MAT * MAT
BTW = 1024  # band buffer pitch (cols 0..512 used)
PPW = 544  # prefix buffer pitch (cols 0..512 used)
HALO = 465  # halo columns: max index 32*15+31*15+31 = 976 -> 976-512+1

LAST_EXEC_TIME_NS = None
_COMPILED = None


def _ensure_axon_ntff_hook():
    """This image's antenv lacks axon_hooks; bass_utils hard-imports it when
    trace=True under axon. Recreate the module and install the ctypes-based
    NTFF hook the boot shim would have installed. Degrades to no-op."""
    try:
        from antenv import axon_hooks  # noqa: F401

        return
    except ImportError:
        pass
    try:
        import types

        import antenv

        m = types.ModuleType("antenv.axon_hooks")
        _hook = [None]
        m.set_axon_ntff_profile_hook = lambda h: _hook.__setitem__(0, h)
        m.get_axon_ntff_profile_hook = lambda: _hook[0]
        sys.modules["antenv.axon_hooks"] = m
        antenv.axon_hooks = m
        if "/root/.axon_site" not in sys.path:
            sys.path.insert(0, "/root/.axon_site")
        from trn_agent_boot import trn_boot

        hook = trn_boot._ntff_profile_via_ctypes("/opt/axon/libaxon_pjrt.so")
        if hook is not None:
            m.set_axon_ntff_profile_hook(hook)
    except Exception:
        pass


def _make_weights() -> np.ndarray:
    """[127, 256] f32: cols 0..125 = (1/256)*I (PE copy of P[q]),
    cols 128..253 = (1/256)*shift (copy of P[q+1]; shift[p,oc]=1 iff p==oc+1)."""
    w = np.zeros((NP, 256), dtype=np.float32)
    q = np.arange(NQ)
    w[q, q] = 1.0 / 256.0
    w[q + 1, 128 + q] = 1.0 / 256.0
    return w


def _build():
    import concourse.bass as bass
    import concourse.bacc as bacc
    from concourse import mybir
    from contextlib import ExitStack

    f32 = mybir.dt.float32
    add = mybir.AluOpType.add
    sub_op = mybir.AluOpType.subtract
    bypass = mybir.AluOpType.bypass
    X = mybir.AxisListType.X

    nc = bacc.Bacc("TRN2", target_bir_lowering=False, debug=False)
    x = nc.dram_tensor("x", [B_PER_CORE, MAT, MAT], f32, kind="ExternalInput")
    w = nc.dram_tensor("w", [NP, 256], f32, kind="ExternalInput")
    y = nc.dram_tensor("y", [B_PER_CORE, ND], f32, kind="ExternalOutput")

    def v(t, off, pat):
        return bass.AP(t, off, pat)

    with ExitStack() as ctx:
        B = B_PER_CORE
        e = ctx.enter_context
        bts = [e(nc.sbuf_tensor(f"bt{i}", [NP, BTW], f32)) for i in range(B)]
        pps = [e(nc.sbuf_tensor(f"pp{i}", [NP, PPW], f32)) for i in range(B)]
        wt = e(nc.sbuf_tensor("wt", [NP, 256], f32))
        ro = e(nc.sbuf_tensor("ro", [NQ, 64], f32))
        rs1 = [e(nc.sbuf_tensor(f"r1_{i}", [NQ, 16], f32)) for i in range(B)]
        rs2 = [e(nc.sbuf_tensor(f"r2_{i}", [NQ, 16], f32)) for i in range(B)]
        pph = [nc.alloc_psum_tensor(f"ph{i}", [NQ, 1024], f32) for i in range(B)]
        bsem = [e(nc.semaphore(f"bsem{i}")) for i in range(B)]
        tsem = [e(nc.semaphore(f"tsem{i}")) for i in range(B)]
        wsem = e(nc.semaphore("wsem"))
        gsem = e(nc.semaphore("gsem"))
        vscan = e(nc.semaphore("vscan"))
        mmsem = e(nc.semaphore("mmsem"))
        acsem = e(nc.semaphore("acsem"))
        vred = e(nc.semaphore("vred"))
        vec_done = e(nc.semaphore("vec_done"))
        dma_out = e(nc.semaphore("dma_out"))
        block = e(nc.Block(no_gpsimd_drain=True))

        @block.sync
        def _(sync):
            for b in range(B):
                # band: bt[p, 1+32t+j] = x[b, 16p+t, 16p+t+2+j], j in [0,32)
                sync.dma_start(
                    v(bts[b], 1, [[BTW, NQ], [32, 16], [1, 32]]),
                    bass.AP(
                        x,
                        b * MAT_ELEMS + 2,
                        [[16 * ROW_STRIDE, NQ], [ROW_STRIDE, 16], [1, 32]],
                    ),
                ).then_inc(bsem[b], 16)
            # merged outputs: y[b, 16q+u] <- ro[q, 16b+u]; batch 3 separate
            # so batches 0-2 overlap the tail
            sync.wait_ge(vec_done, B - 1)
            sync.dma_start(
                bass.AP(y, 0, [[16, NQ], [ND, B - 1], [1, 16]]),
                v(ro, 0, [[64, NQ], [16, B - 1], [1, 16]]),
            ).then_inc(dma_out, 16)
            sync.wait_ge(vec_done, B)
            sync.dma_start(
                bass.AP(y, (B - 1) * ND, [[16, NQ], [1, 16]]),
                v(ro, 16 * (B - 1), [[64, NQ], [1, 16]]),
            ).then_inc(dma_out, 16)
            sync.wait_ge(dma_out, 32)

        @block.scalar
        def _(scalar):
            for b in range(B):
                # partition 126's band rows (halo source for q=125)
                scalar.dma_start(
                    v(bts[b], NQ * BTW + 1, [[BTW, 1], [32, 16], [1, 32]]),
                    bass.AP(
                        x,
                        b * MAT_ELEMS + 2 + NQ * 16 * ROW_STRIDE,
                        [[16 * ROW_STRIDE, 1], [ROW_STRIDE, 16], [1, 32]],
                    ),
                ).then_inc(tsem[b], 16)
            # w-load after the tails: its 258KB stop competing with band 0's
            # early SDMA window (126+1 split keeps the spray-friendly count)
            scalar.dma_start(
                v(wt, 0, [[256, NQ], [1, 256]]),
                bass.AP(w, 0, [[256, NQ], [1, 256]]),
            ).then_inc(wsem, 16)
            scalar.dma_start(
                v(wt, NQ * 256, [[256, 1], [1, 256]]),
                bass.AP(w, NQ * 256, [[256, 1], [1, 256]]),
            ).then_inc(wsem, 16)
            for b in range(B):
                # PPH[q, f] = P[q, f]/256 (PSUM bank 0) on the idle ACT engine
                scalar.wait_ge(vscan, b + 1)
                nc.scalar.activation(
                    out=v(pph[b], 0, [[1024, NQ], [1, 512]]),
                    in_=v(pps[b], 0, [[PPW, NQ], [1, 512]]),
                    func=mybir.ActivationFunctionType.Copy,
                    bias=0.0,
                    scale=1.0 / 256.0,
                ).then_inc(acsem, 1)

        @block.tensor
        def _(tensor):
            # PPH[q, 512+g] = P[q+1, g]/256 (bank 1): the PE's shift matmul
            # replaces the SBUF->SBUF halo DMA; junk warmup matmuls first to
            # release the HAM clock throttle before the real ones arrive.
            tensor.wait_ge(wsem, 32)
            for _ in range(6):
                nc.tensor.matmul(
                    v(pph[0], 512, [[1024, NQ], [1, 64]]),
                    v(wt, 128, [[256, NP], [1, NQ]]),
                    v(wt, 0, [[256, NP], [1, 64]]),
                    start=True,
                    stop=True,
                )
            for b in range(B):
                tensor.wait_ge(vscan, b + 1)
                nc.tensor.matmul(
                    v(pph[b], 512, [[1024, NQ], [1, HALO]]),
                    v(wt, 128, [[256, NP], [1, NQ]]),
                    v(pps[b], 0, [[PPW, NP], [1, HALO]]),
                    start=True,
                    stop=True,
                ).then_inc(mmsem, 1)

        def rblk(vector, b):
            # out[16q+u] = sum_s PPH[32u+31s+31] - sum_s PPH[32u+31s+15]
            vector.wait_ge(mmsem, b + 1)
            vector.wait_ge(acsem, b + 1)
            nc.vector.reduce_sum(
                out=v(rs1[b], 0, [[16, NQ], [1, 16]]),
                in_=v(pph[b], 31, [[1024, NQ], [32, 16], [31, 16]]),
                axis=X,
            ).then_inc(vred, 1)
            nc.vector.reduce_sum(
                out=v(rs2[b], 0, [[16, NQ], [1, 16]]),
                in_=v(pph[b], 15, [[1024, NQ], [32, 16], [31, 16]]),
                axis=X,
            ).then_inc(vred, 1)
            vector.wait_ge(vred, 2 * (b + 1))
            nc.vector.tensor_tensor(
                out=v(ro, 16 * b, [[64, NQ], [1, 16]]),
                in0=v(rs1[b], 0, [[16, NQ], [1, 16]]),
                in1=v(rs2[b], 0, [[16, NQ], [1, 16]]),
                op=sub_op,
            ).then_inc(vec_done, 1)

        @block.vector
        def _(vector):
            for b in range(B):
                # band col 0: never DMA'd; zero so the scan emits P[0] = 0
                nc.vector.memset(
                    v(bts[b], 0, [[BTW, NP], [1, 1]]), 0.0
                ).then_inc(gsem, 1)
            vector.wait_ge(gsem, B)
            for b in range(B):
                vector.wait_ge(bsem[b], 16)
                vector.wait_ge(tsem[b], 16)
                # P[f] = prefix sum of the flat band per partition; P[0] = 0
                nc.vector.tensor_tensor_scan(
                    out=v(pps[b], 0, [[PPW, NP], [1, 513]]),
                    data0=v(bts[b], 0, [[BTW, NP], [1, 513]]),
                    data1=v(bts[b], 0, [[BTW, NP], [1, 513]]),
                    initial=0.0,
                    op0=add,
                    op1=bypass,
                ).then_inc(vscan, 1)
                if b == 3:
                    rblk(vector, 0)
            rblk(vector, 1)
            rblk(vector, 2)
            rblk(vector, 3)

    nc.compile()
    return nc


def _get_compiled():
    global _COMPILED
    if _COMPILED is None:
        _COMPILED = _build()
    return _COMPILED


def kernel(x: np.ndarray) -> np.ndarray:
    global LAST_EXEC_TIME_NS
    from concourse.bass_utils import run_bass_kernel_spmd

    x = np.ascontiguousarray(np.asarray(x), dtype=np.float32)
    assert x.shape == (B_FULL, MAT, MAT), x.shape

    nc = _get_compiled()
    wmat = _make_weights()
    in_maps = [
        {"x": x[i * B_PER_CORE : (i + 1) * B_PER_CORE], "w": wmat}
        for i in range(N_CORES)
    ]
    trace = bool(int(os.environ.get("KERNEL_TRACE", "0")))
    if trace:
        _ensure_axon_ntff_hook()
        # test-only: keep NTFF artifacts local instead of uploading
        from concourse import bass_utils as _bu

        _bu.upload_artifacts = lambda tmpdir: tmpdir
    res = run_bass_kernel_spmd(
        nc, in_maps, core_ids=list(range(N_CORES)), trace=trace
    )
    LAST_EXEC_TIME_NS = res.exec_time_ns
    out = np.concatenate([res.results[i]["y"] for i in range(N_CORES)], axis=0)
    return out.astype(np.float32)


# revision 39
# speedup vs baseline: 3.9100x; 1.0584x over previous
"""Trainium2 Bass kernel for DiamondLayer.

Computes out[b, d] = mean(x[b, d:d+16, d+17:d+33]) for d in [0, 2016):
16x16 mean-pool windows sliding along the diagonal of each 2048x2048 matrix.

Sharding: pure data parallel over batch - 32 batches -> 8 cores x 4 batches.

Per-core kernel (raw bacc, no Tile):
  - Only the diagonal band cols [r+2, r+34) of row r is ever touched, so each
    core DMAs just that band with a strided access pattern: partition p holds
    rows [16p, 16p+16), one 128B run per row (2016+16 descriptors/batch).
    One band DMA per batch on the SP ring (126 partitions - the HWDGE spray
    across 14 SDMA engines needs the outer AP count divisible by 14) plus a
    partition-126 tail DMA on the ACT ring.
  - VectorE computes a per-partition prefix scan P of the flat band
    (tensor_tensor_scan); window sums become differences of P.
  - The halo'd, prescaled prefix buffer PPH = P/256 is built in PSUM by two
    idle engines: ACT copies P[q, 0:512) to bank 0 (activation Copy with
    scale=1/256) and PE copies P[q+1, 0:465) to bank 1 via a matmul with a
    (1/256)*shift-by-one weight matrix (fed as an extra kernel input) -
    replacing the SBUF->SBUF halo DMA, which used to steal SDMA descriptor
    throughput from the band and cost ~3us of tail latency. Junk warmup
    matmuls release the PE HAM clock throttle first.
  - out[16q+u] = sum_s PPH[32u+31s+31] - sum_s PPH[32u+31s+15]: two strided
    DVE reduces + a DVE subtract per batch (1/256 already applied).
  - Two merged output DMAs on the SP ring (batches 0-2, then batch 3).
"""

import os
import sys

import numpy as np

for _p in ("/opt/trn_rl_repo",):
    if _p not in sys.path:
        sys.path.insert(0, _p)

B_FULL = 32
N_CORES = 8
B_PER_CORE = B_FULL // N_CORES  # 4
MAT = 2048
ND = MAT - 32  # 2016
NQ = ND // 16  # 126
NP = NQ + 1  # 127
ROW_STRIDE = MAT + 1  # 2049
MAT_ELEMS = MAT * MAT
BTW = 1024  # band buffer pitch (cols 0..512 used)
PPW = 544  # prefix buffer pitch (cols 0..512 used)
HALO = 465  # halo columns: max index 32*15+31*15+31 = 976 -> 976-512+1

LAST_EXEC_TIME_NS = None
_COMPILED = None


def _ensure_axon_ntff_hook():
    """This image's antenv lacks axon_hooks; bass_utils hard-imports it when
    trace=True under axon. Recreate the module and install the ctypes-based
    NTFF hook the boot shim would have installed. Degrades to no-op."""
    try:
        from antenv import axon_hooks  # noqa: F401

        return
    except ImportError:
        pass
    try:
        import types

        import antenv

        m = types.ModuleType("antenv.axon_hooks")
        _hook = [None]
        m.set_axon_ntff_profile_hook = lambda h: _hook.__setitem__(0, h)
        m.get_axon_ntff_profile_hook = lambda: _hook[0]
        sys.modules["antenv.axon_hooks"] = m
        antenv.axon_hooks = m
        if "/root/.axon_site" not in sys.path:
            sys.path.insert(0, "/root/.axon_site")
        from trn_agent_boot import trn_boot

        hook = trn_boot._ntff_profile_via_ctypes("/opt/axon/libaxon_pjrt.so")
        if hook is not None:
            m.set_axon_ntff_profile_hook(hook)
    except Exception:
        pass


def _make_weights() -> np.ndarray:
    """[127, 256] f32: cols 0..125 = (1/256)*I (PE copy of P[q]),
    cols 128..253 = (1/256)*shift (copy of P[q+1]; shift[p,oc]=1 iff p==oc+1)."""
    w = np.zeros((NP, 256), dtype=np.float32)
    q = np.arange(NQ)
    w[q, q] = 1.0 / 256.0
    w[q + 1, 128 + q] = 1.0 / 256.0
    return w


def _build():
    import concourse.bass as bass
    import concourse.bacc as bacc
    from concourse import mybir
    from contextlib import ExitStack

    f32 = mybir.dt.float32
    add = mybir.AluOpType.add
    sub_op = mybir.AluOpType.subtract
    bypass = mybir.AluOpType.bypass
    X = mybir.AxisListType.X

    nc = bacc.Bacc("TRN2", target_bir_lowering=False, debug=False)
    x = nc.dram_tensor("x", [B_PER_CORE, MAT, MAT], f32, kind="ExternalInput")
    w = nc.dram_tensor("w", [NP, 256], f32, kind="ExternalInput")
    y = nc.dram_tensor("y", [B_PER_CORE, ND], f32, kind="ExternalOutput")

    def v(t, off, pat):
        return bass.AP(t, off, pat)

    with ExitStack() as ctx:
        B = B_PER_CORE
        e = ctx.enter_context
        bts = [e(nc.sbuf_tensor(f"bt{i}", [NP, BTW], f32)) for i in range(B)]
        pps = [e(nc.sbuf_tensor(f"pp{i}", [NP, PPW], f32)) for i in range(B)]
        wt = e(nc.sbuf_tensor("wt", [NP, 256], f32))
        ro = e(nc.sbuf_tensor("ro", [NQ, 64], f32))
        rs1 = [e(nc.sbuf_tensor(f"r1_{i}", [NQ, 16], f32)) for i in range(B)]
        rs2 = [e(nc.sbuf_tensor(f"r2_{i}", [NQ, 16], f32)) for i in range(B)]
        pph = [nc.alloc_psum_tensor(f"ph{i}", [NQ, 1024], f32) for i in range(B)]
        bsem = [e(nc.semaphore(f"bsem{i}")) for i in range(B)]
        tsem = [e(nc.semaphore(f"tsem{i}")) for i in range(B)]
        wsem = e(nc.semaphore("wsem"))
        gsem = e(nc.semaphore("gsem"))
        vscan = e(nc.semaphore("vscan"))
        mmsem = e(nc.semaphore("mmsem"))
        acsem = e(nc.semaphore("acsem"))
        vred = e(nc.semaphore("vred"))
        vec_done = e(nc.semaphore("vec_done"))
        dma_out = e(nc.semaphore("dma_out"))
        block = e(nc.Block(no_gpsimd_drain=True))

        @block.sync
        def _(sync):
            for b in range(B):
                # band: bt[p, 1+32t+j] = x[b, 16p+t, 16p+t+2+j], j in [0,32)
                sync.dma_start(
                    v(bts[b], 1, [[BTW, NQ], [32, 16], [1, 32]]),
                    bass.AP(
                        x,
                        b * MAT_ELEMS + 2,
                        [[16 * ROW_STRIDE, NQ], [ROW_STRIDE, 16], [1, 32]],
                    ),
                ).then_inc(bsem[b], 16)
            # merged outputs: y[b, 16q+u] <- ro[q, 16b+u]; batch 3 separate
            # so batches 0-2 overlap the tail
            sync.wait_ge(vec_done, B - 1)
            sync.dma_start(
                bass.AP(y, 0, [[16, NQ], [ND, B - 1], [1, 16]]),
                v(ro, 0, [[64, NQ], [16, B - 1], [1, 16]]),
            ).then_inc(dma_out, 16)
            sync.wait_ge(vec_done, B)
            sync.dma_start(
                bass.AP(y, (B - 1) * ND, [[16, NQ], [1, 16]]),
                v(ro, 16 * (B - 1), [[64, NQ], [1, 16]]),
            ).then_inc(dma_out, 16)
            sync.wait_ge(dma_out, 32)

        @block.scalar
        def _(scalar):
            # split 126+1 so the big piece's outer count stays spray-friendly
            scalar.dma_start(
                v(wt, 0, [[256, NQ], [1, 256]]),
                bass.AP(w, 0, [[256, NQ], [1, 256]]),
            ).then_inc(wsem, 16)
            scalar.dma_start(
                v(wt, NQ * 256, [[256, 1], [1, 256]]),
                bass.AP(w, NQ * 256, [[256, 1], [1, 256]]),
            ).then_inc(wsem, 16)
            for b in range(B):
                # partition 126's band rows (halo source for q=125)
                scalar.dma_start(
                    v(bts[b], NQ * BTW + 1, [[BTW, 1], [32, 16], [1, 32]]),
                    bass.AP(
                        x,
                        b * MAT_ELEMS + 2 + NQ * 16 * ROW_STRIDE,
                        [[16 * ROW_STRIDE, 1], [ROW_STRIDE, 16], [1, 32]],
                    ),
                ).then_inc(tsem[b], 16)
            for b in range(B):
                # PPH[q, f] = P[q, f]/256 (PSUM bank 0) on the idle ACT engine
                scalar.wait_ge(vscan, b + 1)
                nc.scalar.activation(
                    out=v(pph[b], 0, [[1024, NQ], [1, 512]]),
                    in_=v(pps[b], 0, [[PPW, NQ], [1, 512]]),
                    func=mybir.ActivationFunctionType.Copy,
                    bias=0.0,
                    scale=1.0 / 256.0,
                ).then_inc(acsem, 1)

        @block.tensor
        def _(tensor):
            # PPH[q, 512+g] = P[q+1, g]/256 (bank 1): the PE's shift matmul
            # replaces the SBUF->SBUF halo DMA; junk warmup matmuls first to
            # release the HAM clock throttle before the real ones arrive.
            tensor.wait_ge(wsem, 32)
            for _ in range(6):
                nc.tensor.matmul(
                    v(pph[0], 512, [[1024, NQ], [1, 64]]),
                    v(wt, 128, [[256, NP], [1, NQ]]),
                    v(wt, 0, [[256, NP], [1, 64]]),
                    start=True,
                    stop=True,
                )
            for b in range(B):
                tensor.wait_ge(vscan, b + 1)
                nc.tensor.matmul(
                    v(pph[b], 512, [[1024, NQ], [1, HALO]]),
                    v(wt, 128, [[256, NP], [1, NQ]]),
                    v(pps[b], 0, [[PPW, NP], [1, HALO]]),
                    start=True,
                    stop=True,
                ).then_inc(mmsem, 1)

        def rblk(vector, b):
            # out[16q+u] = sum_s PPH[32u+31s+31] - sum_s PPH[32u+31s+15]
            vector.wait_ge(mmsem, b + 1)
            vector.wait_ge(acsem, b + 1)
            nc.vector.reduce_sum(
                out=v(rs1[b], 0, [[16, NQ], [1, 16]]),
                in_=v(pph[b], 31, [[1024, NQ], [32, 16], [31, 16]]),
                axis=X,
            ).then_inc(vred, 1)
            nc.vector.reduce_sum(
                out=v(rs2[b], 0, [[16, NQ], [1, 16]]),
                in_=v(pph[b], 15, [[1024, NQ], [32, 16], [31, 16]]),
                axis=X,
            ).then_inc(vred, 1)
            vector.wait_ge(vred, 2 * (b + 1))
            nc.vector.tensor_tensor(
                out=v(ro, 16 * b, [[64, NQ], [1, 16]]),
                in0=v(rs1[b], 0, [[16, NQ], [1, 16]]),
                in1=v(rs2[b], 0, [[16, NQ], [1, 16]]),
                op=sub_op,
            ).then_inc(vec_done, 1)

        @block.vector
        def _(vector):
            for b in range(B):
                # band col 0: never DMA'd; zero so the scan emits P[0] = 0
                nc.vector.memset(
                    v(bts[b], 0, [[BTW, NP], [1, 1]]), 0.0
                ).then_inc(gsem, 1)
            vector.wait_ge(gsem, B)
            for b in range(B):
                vector.wait_ge(bsem[b], 16)
                vector.wait_ge(tsem[b], 16)
                # P[f] = prefix sum of the flat band per partition; P[0] = 0
                nc.vector.tensor_tensor_scan(
                    out=v(pps[b], 0, [[PPW, NP], [1, 513]]),
                    data0=v(bts[b], 0, [[BTW, NP], [1, 513]]),
                    data1=v(bts[b], 0, [[BTW, NP], [1, 513]]),
                    initial=0.0,
                    op0=add,
                    op1=bypass,
                ).then_inc(vscan, 1)
                if b == 3:
                    rblk(vector, 0)
            rblk(vector, 1)
            rblk(vector, 2)
            rblk(vector, 3)

    nc.compile()
    return nc


def _get_compiled():
    global _COMPILED
    if _COMPILED is None:
        _COMPILED = _build()
    return _COMPILED


def kernel(x: np.ndarray) -> np.ndarray:
    global LAST_EXEC_TIME_NS
    from concourse.bass_utils import run_bass_kernel_spmd

    x = np.ascontiguousarray(np.asarray(x), dtype=np.float32)
    assert x.shape == (B_FULL, MAT, MAT), x.shape

    nc = _get_compiled()
    wmat = _make_weights()
    in_maps = [
        {"x": x[i * B_PER_CORE : (i + 1) * B_PER_CORE], "w": wmat}
        for i in range(N_CORES)
    ]
    trace = bool(int(os.environ.get("KERNEL_TRACE", "0")))
    if trace:
        _ensure_axon_ntff_hook()
        # test-only: keep NTFF artifacts local instead of uploading
        from concourse import bass_utils as _bu

        _bu.upload_artifacts = lambda tmpdir: tmpdir
    res = run_bass_kernel_spmd(
        nc, in_maps, core_ids=list(range(N_CORES)), trace=trace
    )
    LAST_EXEC_TIME_NS = res.exec_time_ns
    out = np.concatenate([res.results[i]["y"] for i in range(N_CORES)], axis=0)
    return out.astype(np.float32)


# revision 40
# speedup vs baseline: 3.9405x; 1.0078x over previous
"""Trainium2 Bass kernel for DiamondLayer.

Computes out[b, d] = mean(x[b, d:d+16, d+17:d+33]) for d in [0, 2016):
16x16 mean-pool windows sliding along the diagonal of each 2048x2048 matrix.

Sharding: pure data parallel over batch - 32 batches -> 8 cores x 4 batches.

Per-core kernel (raw bacc, no Tile):
  - Only the diagonal band cols [r+2, r+34) of row r is ever touched, so each
    core DMAs just that band with a strided access pattern: partition p holds
    rows [16p, 16p+16), one 128B run per row (2016+16 descriptors/batch).
    One band DMA per batch on the SP ring (126 partitions - the HWDGE spray
    across 14 SDMA engines needs the outer AP count divisible by 14) plus a
    partition-126 tail DMA on the ACT ring.
  - VectorE computes a per-partition prefix scan P of the flat band
    (tensor_tensor_scan); window sums become differences of P.
  - The halo'd, prescaled prefix buffer PPH = P/256 is built in PSUM by two
    idle engines: ACT copies P[q, 0:512) to bank 0 (activation Copy with
    scale=1/256) and PE copies P[q+1, 0:465) to bank 1 via a matmul with a
    (1/256)*shift-by-one weight matrix (fed as an extra kernel input) -
    replacing the SBUF->SBUF halo DMA, which used to steal SDMA descriptor
    throughput from the band and cost ~3us of tail latency. Junk warmup
    matmuls release the PE HAM clock throttle first.
  - out[16q+u] = sum_s PPH[32u+31s+31] - sum_s PPH[32u+31s+15]: two strided
    DVE reduces + a DVE subtract per batch (1/256 already applied).
  - Two merged output DMAs on the SP ring (batches 0-2, then batch 3).
"""

import os
import sys

import numpy as np

for _p in ("/opt/trn_rl_repo",):
    if _p not in sys.path:
        sys.path.insert(0, _p)

B_FULL = 32
N_CORES = 8
B_PER_CORE = B_FULL // N_CORES  # 4
MAT = 2048
ND = MAT - 32  # 2016
NQ = ND // 16  # 126
NP = NQ + 1  # 127
ROW_STRIDE = MAT + 1  # 2049
MAT_ELEMS = MAT * MAT
BTW = 1024  # band buffer pitch (cols 0..512 used)
PPW = 544  # prefix buffer pitch (cols 0..512 used)
HALO = 465  # halo columns: max index 32*15+31*15+31 = 976 -> 976-512+1

LAST_EXEC_TIME_NS = None
_COMPILED = None


def _ensure_axon_ntff_hook():
    """This image's antenv lacks axon_hooks; bass_utils hard-imports it when
    trace=True under axon. Recreate the module and install the ctypes-based
    NTFF hook the boot shim would have installed. Degrades to no-op."""
    try:
        from antenv import axon_hooks  # noqa: F401

        return
    except ImportError:
        pass
    try:
        import types

        import antenv

        m = types.ModuleType("antenv.axon_hooks")
        _hook = [None]
        m.set_axon_ntff_profile_hook = lambda h: _hook.__setitem__(0, h)
        m.get_axon_ntff_profile_hook = lambda: _hook[0]
        sys.modules["antenv.axon_hooks"] = m
        antenv.axon_hooks = m
        if "/root/.axon_site" not in sys.path:
            sys.path.insert(0, "/root/.axon_site")
        from trn_agent_boot import trn_boot

        hook = trn_boot._ntff_profile_via_ctypes("/opt/axon/libaxon_pjrt.so")
        if hook is not None:
            m.set_axon_ntff_profile_hook(hook)
    except Exception:
        pass


def _make_weights() -> np.ndarray:
    """[127, 256] f32: cols 0..125 = (1/256)*I (PE copy of P[q]),
    cols 128..253 = (1/256)*shift (copy of P[q+1]; shift[p,oc]=1 iff p==oc+1)."""
    w = np.zeros((NP, 256), dtype=np.float32)
    q = np.arange(NQ)
    w[q, q] = 1.0 / 256.0
    w[q + 1, 128 + q] = 1.0 / 256.0
    return w


def _build():
    import concourse.bass as bass
    import concourse.bacc as bacc
    from concourse import mybir
    from contextlib import ExitStack

    f32 = mybir.dt.float32
    add = mybir.AluOpType.add
    sub_op = mybir.AluOpType.subtract
    bypass = mybir.AluOpType.bypass
    X = mybir.AxisListType.X

    nc = bacc.Bacc("TRN2", target_bir_lowering=False, debug=False)
    x = nc.dram_tensor("x", [B_PER_CORE, MAT, MAT], f32, kind="ExternalInput")
    w = nc.dram_tensor("w", [NP, 256], f32, kind="ExternalInput")
    y = nc.dram_tensor("y", [B_PER_CORE, ND], f32, kind="ExternalOutput")

    def v(t, off, pat):
        return bass.AP(t, off, pat)

    with ExitStack() as ctx:
        B = B_PER_CORE
        e = ctx.enter_context
        bts = [e(nc.sbuf_tensor(f"bt{i}", [NP, BTW], f32)) for i in range(B)]
        pps = [e(nc.sbuf_tensor(f"pp{i}", [NP, PPW], f32)) for i in range(B)]
        wt = e(nc.sbuf_tensor("wt", [NP, 256], f32))
        ro = e(nc.sbuf_tensor("ro", [NQ, 64], f32))
        rs1 = [e(nc.sbuf_tensor(f"r1_{i}", [NQ, 16], f32)) for i in range(B)]
        rs2 = [e(nc.sbuf_tensor(f"r2_{i}", [NQ, 16], f32)) for i in range(B)]
        pph = [nc.alloc_psum_tensor(f"ph{i}", [NQ, 1024], f32) for i in range(B)]
        bsem = [e(nc.semaphore(f"bsem{i}")) for i in range(B)]
        tsem = [e(nc.semaphore(f"tsem{i}")) for i in range(B)]
        wsem = e(nc.semaphore("wsem"))
        gsem = e(nc.semaphore("gsem"))
        vscan = e(nc.semaphore("vscan"))
        mmsem = e(nc.semaphore("mmsem"))
        acsem = e(nc.semaphore("acsem"))
        vred = e(nc.semaphore("vred"))
        psem = e(nc.semaphore("psem"))
        vec_done = e(nc.semaphore("vec_done"))
        dma_out = e(nc.semaphore("dma_out"))
        block = e(nc.Block(no_gpsimd_drain=True))

        @block.sync
        def _(sync):
            for b in range(B):
                # band: bt[p, 1+32t+j] = x[b, 16p+t, 16p+t+2+j], j in [0,32)
                sync.dma_start(
                    v(bts[b], 1, [[BTW, NQ], [32, 16], [1, 32]]),
                    bass.AP(
                        x,
                        b * MAT_ELEMS + 2,
                        [[16 * ROW_STRIDE, NQ], [ROW_STRIDE, 16], [1, 32]],
                    ),
                ).then_inc(bsem[b], 16)
            # merged outputs: y[b, 16q+u] <- ro[q, 16b+u]; batch 3 separate
            # so batches 0-2 overlap the tail
            sync.wait_ge(psem, B - 1)
            sync.dma_start(
                bass.AP(y, 0, [[16, NQ], [ND, B - 1], [1, 16]]),
                v(ro, 0, [[64, NQ], [16, B - 1], [1, 16]]),
            ).then_inc(dma_out, 16)
            sync.wait_ge(psem, B - 1)
            sync.wait_ge(vec_done, 1)
            sync.dma_start(
                bass.AP(y, (B - 1) * ND, [[16, NQ], [1, 16]]),
                v(ro, 16 * (B - 1), [[64, NQ], [1, 16]]),
            ).then_inc(dma_out, 16)
            sync.wait_ge(dma_out, 32)

        @block.scalar
        def _(scalar):
            # split 126+1 so the big piece's outer count stays spray-friendly
            scalar.dma_start(
                v(wt, 0, [[256, NQ], [1, 256]]),
                bass.AP(w, 0, [[256, NQ], [1, 256]]),
            ).then_inc(wsem, 16)
            scalar.dma_start(
                v(wt, NQ * 256, [[256, 1], [1, 256]]),
                bass.AP(w, NQ * 256, [[256, 1], [1, 256]]),
            ).then_inc(wsem, 16)
            for b in range(B):
                # partition 126's band rows (halo source for q=125)
                scalar.dma_start(
                    v(bts[b], NQ * BTW + 1, [[BTW, 1], [32, 16], [1, 32]]),
                    bass.AP(
                        x,
                        b * MAT_ELEMS + 2 + NQ * 16 * ROW_STRIDE,
                        [[16 * ROW_STRIDE, 1], [ROW_STRIDE, 16], [1, 32]],
                    ),
                ).then_inc(tsem[b], 16)
            for b in range(B):
                # PPH[q, f] = P[q, f]/256 (PSUM bank 0) on the idle ACT engine
                scalar.wait_ge(vscan, b + 1)
                nc.scalar.activation(
                    out=v(pph[b], 0, [[1024, NQ], [1, 512]]),
                    in_=v(pps[b], 0, [[PPW, NQ], [1, 512]]),
                    func=mybir.ActivationFunctionType.Copy,
                    bias=0.0,
                    scale=1.0 / 256.0,
                ).then_inc(acsem, 1)

        @block.tensor
        def _(tensor):
            # PPH[q, 512+g] = P[q+1, g]/256 (bank 1): the PE's shift matmul
            # replaces the SBUF->SBUF halo DMA; junk warmup matmuls first to
            # release the HAM clock throttle before the real ones arrive.
            tensor.wait_ge(wsem, 32)
            for _ in range(6):
                nc.tensor.matmul(
                    v(pph[0], 512, [[1024, NQ], [1, 64]]),
                    v(wt, 128, [[256, NP], [1, NQ]]),
                    v(wt, 0, [[256, NP], [1, 64]]),
                    start=True,
                    stop=True,
                )
            for b in range(B):
                tensor.wait_ge(vscan, b + 1)
                nc.tensor.matmul(
                    v(pph[b], 512, [[1024, NQ], [1, HALO]]),
                    v(wt, 128, [[256, NP], [1, NQ]]),
                    v(pps[b], 0, [[PPW, NP], [1, HALO]]),
                    start=True,
                    stop=True,
                ).then_inc(mmsem, 1)

        @block.gpsimd
        def _(gpsimd):
            for b in range(B - 1):
                # out[:, 16b..] = R1 - R2 for batches 0-2 on the idle Pool
                gpsimd.wait_ge(vred, 2 * (b + 1))
                nc.gpsimd.tensor_tensor(
                    out=v(ro, 16 * b, [[64, NQ], [1, 16]]),
                    in0=v(rs1[b], 0, [[16, NQ], [1, 16]]),
                    in1=v(rs2[b], 0, [[16, NQ], [1, 16]]),
                    op=sub_op,
                ).then_inc(psem, 1)

        def rblk(vector, b):
            # out[16q+u] = sum_s PPH[32u+31s+31] - sum_s PPH[32u+31s+15]
            vector.wait_ge(mmsem, b + 1)
            vector.wait_ge(acsem, b + 1)
            nc.vector.reduce_sum(
                out=v(rs1[b], 0, [[16, NQ], [1, 16]]),
                in_=v(pph[b], 31, [[1024, NQ], [32, 16], [31, 16]]),
                axis=X,
            ).then_inc(vred, 1)
            nc.vector.reduce_sum(
                out=v(rs2[b], 0, [[16, NQ], [1, 16]]),
                in_=v(pph[b], 15, [[1024, NQ], [32, 16], [31, 16]]),
                axis=X,
            ).then_inc(vred, 1)
            if b == B - 1:
                vector.wait_ge(vred, 2 * (b + 1))
                nc.vector.tensor_tensor(
                    out=v(ro, 16 * b, [[64, NQ], [1, 16]]),
                    in0=v(rs1[b], 0, [[16, NQ], [1, 16]]),
                    in1=v(rs2[b], 0, [[16, NQ], [1, 16]]),
                    op=sub_op,
                ).then_inc(vec_done, 1)

        @block.vector
        def _(vector):
            for b in range(B):
                # band col 0: never DMA'd; zero so the scan emits P[0] = 0
                nc.vector.memset(
                    v(bts[b], 0, [[BTW, NP], [1, 1]]), 0.0
                ).then_inc(gsem, 1)
            vector.wait_ge(gsem, B)
            for b in range(B):
                vector.wait_ge(bsem[b], 16)
                vector.wait_ge(tsem[b], 16)
                # P[f] = prefix sum of the flat band per partition; P[0] = 0
                nc.vector.tensor_tensor_scan(
                    out=v(pps[b], 0, [[PPW, NP], [1, 513]]),
                    data0=v(bts[b], 0, [[BTW, NP], [1, 513]]),
                    data1=v(bts[b], 0, [[BTW, NP], [1, 513]]),
                    initial=0.0,
                    op0=add,
                    op1=bypass,
                ).then_inc(vscan, 1)
                if b == 3:
                    rblk(vector, 0)
            rblk(vector, 1)
            rblk(vector, 2)
            rblk(vector, 3)

    nc.compile()
    return nc


def _get_compiled():
    global _COMPILED
    if _COMPILED is None:
        _COMPILED = _build()
    return _COMPILED


def kernel(x: np.ndarray) -> np.ndarray:
    global LAST_EXEC_TIME_NS
    from concourse.bass_utils import run_bass_kernel_spmd

    x = np.ascontiguousarray(np.asarray(x), dtype=np.float32)
    assert x.shape == (B_FULL, MAT, MAT), x.shape

    nc = _get_compiled()
    wmat = _make_weights()
    in_maps = [
        {"x": x[i * B_PER_CORE : (i + 1) * B_PER_CORE], "w": wmat}
        for i in range(N_CORES)
    ]
    trace = bool(int(os.environ.get("KERNEL_TRACE", "0")))
    if trace:
        _ensure_axon_ntff_hook()
        # test-only: keep NTFF artifacts local instead of uploading
        from concourse import bass_utils as _bu

        _bu.upload_artifacts = lambda tmpdir: tmpdir
    res = run_bass_kernel_spmd(
        nc, in_maps, core_ids=list(range(N_CORES)), trace=trace
    )
    LAST_EXEC_TIME_NS = res.exec_time_ns
    out = np.concatenate([res.results[i]["y"] for i in range(N_CORES)], axis=0)
    return out.astype(np.float32)
